# revision 20
# baseline (speedup 1.0000x reference)
"""GCMCGraphConv Trainium2 kernel (8 NeuronCores, SPMD).

Design notes (v3):

Sharding: destination-partitioned edge parallelism. Edges are sorted by
edge_dst on the host; consecutive nonzero-degree dst rows are greedily
packed into groups of <=CAP edges and <=128 rows, and groups are dealt
round-robin to the 8 cores. Every group has a fixed capacity of CAP
slots (3 tiles of 128), so the SPMD program is identical across cores;
padding slots carry zero weights.

The per-edge table rows are pre-gathered ON THE HOST into dense per-slot
streams (the SWDGE indirect-DMA path costs ~1us of gpsimd time per 128
gathered rows on this target, which would dominate the kernel; dense
streams move the same bytes at full DMA bandwidth):
  - rth  [128, n_slots] bf16: review embedding rows, pre-TRANSPOSED so
    the MLP's first matmul can consume them directly (no on-chip
    transposes).
  - fth  [n_slots, 256] bf16: feat rows per slot, pre-scaled by
    wpa = sigmoid(rfeat@prob_w)*cj*ci on the host.
The other gating scalar wra = sigmoid(rfeat@score_w)*cj*ci ships as a
per-slot weight and scales the one-hot scatter matrix of the a2 path.

On-chip per tile (128 slots): MLP layer1 (bf16) + Gelu -> fp8e5, layer2
as ONE DoubleRow fp8 matmul (256-deep contraction in half the cycles) +
Gelu -> bf16. Layer3 (rw3) is NOT applied per edge: messages are
scatter-summed first and rw3 is folded into the final linear
(out_rf = (lin_w@rw3) @ G with G = sum_e wra_e * a2_e one-hot scattered,
which is exact because everything after the second Gelu is linear).

Scatter: per tile a plain one-hot S = onehot(dst) and its scaled copy
S_b = S*wra (one single-op DVE instruction each) feed four 128-col matmuls
accumulating hTa = sum wpa*feat (x) onehot and G = sum wra*a2 (x) onehot
in PSUM over the group's 3 tiles. Per group, four 256-col matmuls apply
lin_w / lin_w@rw3 to produce out[dst0:dst0+128, 256]; the bias and the
zero-degree rows are applied on the host during reassembly.
"""

import os

import numpy as np

P = 128
FEAT = 256
REV_DIM = 128
CAP = 384        # slots per group (3 tiles), <=128 dst rows per group
TPG = CAP // P   # tiles per group = 3
GB = 8           # groups per gather/stage batch
N_CORES = 8

_prog_cache = {}


def _build_program(ng):
    from concourse import tile, mybir, bacc

    n_slots = ng * CAP
    T = ng * TPG                 # tiles per core
    nb = ng // GB                # stage batches
    SLOTS_B = CAP * GB           # slots per batch
    TB = TPG * GB                # tiles per batch
    PAIRS_B = TB // 2
    f32 = mybir.dt.float32
    bf16 = mybir.dt.bfloat16
    f16 = mybir.dt.float16
    fp8 = mybir.dt.float8e5
    MM = mybir.MatmulPerfMode

    nc = bacc.Bacc(None, target_bir_lowering=False, debug=False)

    rth = nc.declare_dram_parameter("rth", [P, n_slots], bf16, isOutput=False)
    fth = nc.declare_dram_parameter("fth", [n_slots, FEAT], bf16, isOutput=False)
    wra = nc.declare_dram_parameter("wra", [P, T], f32, isOutput=False)
    dstr = nc.declare_dram_parameter("dstr", [P, T], f32, isOutput=False)
    rw1t = nc.declare_dram_parameter("rw1t", [REV_DIM, FEAT], bf16, isOutput=False)
    rw2t8 = nc.declare_dram_parameter("rw2t8", [P, 2, FEAT], fp8, isOutput=False)
    w3lt = nc.declare_dram_parameter("w3lt", [P, 2, FEAT], bf16, isOutput=False)
    lwt = nc.declare_dram_parameter("lwt", [P, 2, FEAT], bf16, isOutput=False)
    iota = nc.declare_dram_parameter("iota", [P, P], bf16, isOutput=False)
    outd = nc.declare_dram_parameter("outd", [ng * P, FEAT], bf16, isOutput=True)

    AF = mybir.ActivationFunctionType
    OP = mybir.AluOpType

    with tile.TileContext(nc) as tc:
        with tc.tile_pool(name="const", bufs=1) as cpool, \
             tc.tile_pool(name="stage", bufs=2) as stg, \
             tc.tile_pool(name="a1p", bufs=3) as a1pool, \
             tc.tile_pool(name="msg", bufs=8) as msg, \
             tc.tile_pool(name="drain", bufs=4) as drn, \
             tc.tile_pool(name="ot", bufs=2) as otp, \
             tc.tile_pool(name="psA", bufs=1, space="PSUM") as psA, \
             tc.tile_pool(name="psB", bufs=1, space="PSUM") as psB, \
             tc.tile_pool(name="psC", bufs=3, space="PSUM") as psC, \
             tc.tile_pool(name="psO", bufs=1, space="PSUM") as psO:

            c_rw1t = cpool.tile([REV_DIM, FEAT], bf16)
            nc.sync.dma_start(out=c_rw1t[:], in_=rw1t[:])
            c_rw2t8 = cpool.tile([P, 2, FEAT], fp8)
            nc.sync.dma_start(out=c_rw2t8[:], in_=rw2t8[:])
            c_w3lt = cpool.tile([P, 2, FEAT], bf16)
            nc.sync.dma_start(out=c_w3lt[:], in_=w3lt[:])
            c_lwt = cpool.tile([P, 2, FEAT], bf16)
            nc.sync.dma_start(out=c_lwt[:], in_=lwt[:])
            c_iota = cpool.tile([P, P], bf16)
            nc.sync.dma_start(out=c_iota[:], in_=iota[:])
            c_wra = cpool.tile([P, T], f32)
            nc.sync.dma_start(out=c_wra[:], in_=wra[:])
            c_dstr = cpool.tile([P, T], f32)
            nc.sync.dma_start(out=c_dstr[:], in_=dstr[:])

            for b in range(nb):
                s0 = b * SLOTS_B
                rts = stg.tile([P, SLOTS_B], bf16, tag="rts")
                nc.sync.dma_start(out=rts[:], in_=rth[:, s0:s0 + SLOTS_B])
                fts = stg.tile([P, TB, FEAT], bf16, tag="fts")
                nc.sync.dma_start(
                    out=fts[:],
                    in_=fth[s0:s0 + SLOTS_B, :].rearrange("(n p) d -> p n d", p=P))
                a2b = stg.tile([P, TB, FEAT], bf16, tag="a2b")

                def emit_quad(q):
                    # 4 tiles (512 slots) per emission: wide acts amortize
                    # the activation-engine init overhead; psA/psB are
                    # 2-bank quad tiles, single-buffered.
                    a1ps = psA.tile([P, 2, 2 * FEAT], f32, tag="a1ps")
                    for m in range(2):
                        nc.tensor.matmul(
                            out=a1ps[:, m, :],
                            lhsT=c_rw1t[:, m * P:(m + 1) * P],
                            rhs=rts[:, q * 512:(q + 1) * 512],
                            start=True, stop=True)
                    a1sb = a1pool.tile([P, 2, 2 * FEAT], fp8, tag="a1sb")
                    nc.scalar.activation(out=a1sb[:], in_=a1ps[:], func=AF.Gelu)
                    a2ps = psB.tile([P, 4, FEAT], f32, tag="a2ps")
                    for k in range(4):
                        nc.tensor.matmul(
                            out=a2ps[:, k, :],
                            lhsT=a1sb[:, :, k * P:(k + 1) * P],
                            rhs=c_rw2t8[:],
                            start=True, stop=True, perf_mode=MM.DoubleRow)
                    nc.scalar.activation(out=a2b[:, q * 4:(q + 1) * 4, :],
                                         in_=a2ps[:], func=AF.Gelu)

                po = None

                def emit_group(gi, po):
                    g = b * GB + gi
                    acc = psC.tile([P, 2, FEAT], f32, tag="acc")
                    # PSUM accumulation chains must be contiguous per bank
                    # region on HW: build all S tiles first, then run the
                    # four region chains back to back.
                    sas, sbs = [], []
                    for gt in range(TPG):
                        tg = b * TB + gi * TPG + gt
                        sa = msg.tile([P, P], bf16, tag="sa")
                        nc.vector.tensor_scalar(
                            out=sa[:], in0=c_iota[:],
                            scalar1=c_dstr[:, tg:tg + 1],
                            scalar2=None, op0=OP.is_equal)
                        sas.append(sa)
                        sb_ = msg.tile([P, P], bf16, tag="sb")
                        nc.vector.tensor_scalar(
                            out=sb_[:], in0=sa[:],
                            scalar1=c_wra[:, tg:tg + 1],
                            scalar2=None, op0=OP.mult)
                        sbs.append(sb_)
                    for f in range(2):
                        for gt in range(TPG):
                            ti = gi * TPG + gt
                            nc.tensor.matmul(
                                out=acc[:, 0, f * P:(f + 1) * P],
                                lhsT=fts[:, ti, f * P:(f + 1) * P],
                                rhs=sas[gt][:],
                                start=(gt == 0), stop=(gt == TPG - 1))
                    for j in range(2):
                        for gt in range(TPG):
                            ti = gi * TPG + gt
                            nc.tensor.matmul(
                                out=acc[:, 1, j * P:(j + 1) * P],
                                lhsT=a2b[:, ti, j * P:(j + 1) * P],
                                rhs=sbs[gt][:],
                                start=(gt == 0), stop=(gt == TPG - 1))
                    hg = drn.tile([P, 2, FEAT], bf16, tag="hg")
                    nc.vector.tensor_copy(out=hg[:], in_=acc[:])
                    hta = hg[:, 0, :]
                    gsb = hg[:, 1, :]
                    if gi % 2 == 0:
                        po = psO.tile([P, 2, FEAT], f32, tag="po")
                    for f in range(2):
                        nc.tensor.matmul(
                            out=po[:, gi % 2, :],
                            lhsT=hta[:, f * P:(f + 1) * P],
                            rhs=c_lwt[:, f, :],
                            start=(f == 0), stop=False)
                    for j in range(2):
                        nc.tensor.matmul(
                            out=po[:, gi % 2, :],
                            lhsT=gsb[:, j * P:(j + 1) * P],
                            rhs=c_w3lt[:, j, :],
                            start=False, stop=(j == 1))
                    if gi % 2 == 1:
                        ot = otp.tile([P, 2, FEAT], bf16, tag="ot")
                        nc.scalar.activation(out=ot[:], in_=po[:], func=AF.Copy)
                        g0 = g - 1
                        nc.sync.dma_start(
                            out=outd[g0 * P:(g0 + 2) * P, :].rearrange(
                                "(n p) d -> p n d", p=P),
                            in_=ot[:])
                    return po

                # interleave MLP quads with scatter groups: emit each
                # group as soon as its 3 tiles are computed
                gnext = 0
                for q in range(TB // 4):
                    emit_quad(q)
                    while gnext < GB and (gnext * TPG + TPG - 1) < 4 * (q + 1):
                        po = emit_group(gnext, po)
                        gnext += 1
                while gnext < GB:
                    po = emit_group(gnext, po)
                    gnext += 1
    nc.compile()
    return nc


def kernel(**inputs):
    import ml_dtypes
    from concourse.bass_utils import run_bass_kernel_spmd

    feat = np.asarray(inputs["feat"], dtype=np.float32)
    cj = np.asarray(inputs["cj"], dtype=np.float32)
    ci = np.asarray(inputs["ci"], dtype=np.float32)
    edge_src = np.asarray(inputs["edge_src"]).astype(np.int64)
    edge_dst = np.asarray(inputs["edge_dst"]).astype(np.int64)
    review_id = np.asarray(inputs["review_id"]).astype(np.int64)
    rev_emb = np.asarray(inputs["review_emb"], dtype=np.float32)
    prob_w = np.asarray(inputs["prob_w"], dtype=np.float32)
    score_w = np.asarray(inputs["score_w"], dtype=np.float32)
    rw1 = np.asarray(inputs["rw1"], dtype=np.float32)
    rw2 = np.asarray(inputs["rw2"], dtype=np.float32)
    rw3 = np.asarray(inputs["rw3"], dtype=np.float32)
    lin_w = np.asarray(inputs["lin_w"], dtype=np.float32)
    lin_b = np.asarray(inputs["lin_b"], dtype=np.float32)

    n_dst = ci.shape[0]
    bf = ml_dtypes.bfloat16

    order = np.argsort(edge_dst, kind="stable")
    s_src = edge_src[order]
    s_dst = edge_dst[order]
    s_rev = review_id[order]
    s_w = (cj[s_src, 0] * ci[s_dst, 0]).astype(np.float32)
    rfeat = rev_emb[s_rev]
    pa = 1.0 / (1.0 + np.exp(-(rfeat @ prob_w[0])))
    ra = 1.0 / (1.0 + np.exp(-(rfeat @ score_w[0])))
    s_wpa = (pa * s_w).astype(np.float32)
    s_wra = (ra * s_w).astype(np.float32)

    # nonzero dst rows in sorted order, with degree and edge offsets
    uniq_dst, deg = np.unique(s_dst, return_counts=True)
    row_end = np.cumsum(deg)            # edge offset after each row
    row_start = row_end - deg
    nrows_tot = len(uniq_dst)

    # greedy pack consecutive rows into groups: <=CAP edges, <=128 rows
    grp_row0 = []      # first row index (into uniq_dst) of each group
    grp_nrows = []
    i = 0
    while i < nrows_tot:
        base = row_start[i]
        # farthest j with row_end[j] - base <= CAP
        j = np.searchsorted(row_end, base + CAP, side="right") - 1
        j = min(max(j, i), i + P - 1)
        grp_row0.append(i)
        grp_nrows.append(j - i + 1)
        i = j + 1
    ngroups = len(grp_row0)
    ng = -(-ngroups // N_CORES)
    ng = -(-ng // GB) * GB             # groups per core, multiple of GB
    n_slots = ng * CAP
    T = ng * TPG

    # deal groups round-robin to cores: group g -> core g % 8, index g // 8
    rev16 = rev_emb.astype(bf)
    feat16 = feat.astype(bf)

    consts = dict(
        rw1t=np.ascontiguousarray(rw1.T).astype(bf),
        rw2t8=np.ascontiguousarray(
            rw2.T.reshape(2, P, FEAT).transpose(1, 0, 2)).astype(ml_dtypes.float8_e5m2),
        w3lt=np.ascontiguousarray(
            (lin_w @ rw3).T.reshape(2, P, FEAT).transpose(1, 0, 2)).astype(bf),
        lwt=np.ascontiguousarray(
            lin_w.T.reshape(2, P, FEAT).transpose(1, 0, 2)).astype(bf),
        iota=np.broadcast_to(np.arange(P), (P, P)).astype(bf).copy(),
    )

    in_maps = []
    core_meta = []
    for c in range(N_CORES):
        gl = list(range(c, ngroups, N_CORES))[:ng]
        slot_rev = np.zeros(n_slots, dtype=np.int64)
        slot_src = np.zeros(n_slots, dtype=np.int64)
        wpa_s = np.zeros(n_slots, dtype=np.float32)
        wra_s = np.zeros(n_slots, dtype=np.float32)
        dst_s = np.full(n_slots, -1.0, dtype=np.float32)
        rows_all = []
        pos_all = []
        for k, g in enumerate(gl):
            r0, nr = grp_row0[g], grp_nrows[g]
            e0, e1 = row_start[r0], row_end[r0 + nr - 1]
            n = e1 - e0
            s0 = k * CAP
            slot_rev[s0:s0 + n] = s_rev[e0:e1]
            slot_src[s0:s0 + n] = s_src[e0:e1]
            wpa_s[s0:s0 + n] = s_wpa[e0:e1]
            wra_s[s0:s0 + n] = s_wra[e0:e1]
            dst_s[s0:s0 + n] = (
                np.searchsorted(uniq_dst[r0:r0 + nr], s_dst[e0:e1])
            ).astype(np.float32)
            rows_all.append(uniq_dst[r0:r0 + nr])
            pos_all.append(k * P + np.arange(nr))
        rth = np.ascontiguousarray(rev16[slot_rev].T)           # [128, n_slots]
        fth = np.ascontiguousarray(
            (feat[slot_src] * wpa_s[:, None]).astype(bf))       # [n_slots, 256]
        im = dict(
            rth=rth, fth=fth,
            wra=np.ascontiguousarray(wra_s.reshape(T, P).T),
            dstr=np.ascontiguousarray(dst_s.reshape(T, P).T),
            **consts)
        in_maps.append(im)
        core_meta.append((np.concatenate(rows_all) if rows_all else np.zeros(0, dtype=np.int64),
                          np.concatenate(pos_all) if pos_all else np.zeros(0, dtype=np.int64)))

    global last_inmaps, last_meta
    last_inmaps = in_maps
    last_meta = dict(ng=ng, T=T, n_slots=n_slots, core_meta=core_meta,
                     uniq_dst=uniq_dst, grp_row0=grp_row0, grp_nrows=grp_nrows)

    if ng not in _prog_cache:
        _prog_cache[ng] = _build_program(ng)
    nc = _prog_cache[ng]

    trace = bool(os.environ.get("BASS_KERNEL_TRACE"))
    res = run_bass_kernel_spmd(nc, in_maps, core_ids=list(range(N_CORES)),
                               trace=trace)
    global last_results
    last_results = res

    out = np.broadcast_to(lin_b, (n_dst, FEAT)).astype(np.float32).copy()
    for c in range(N_CORES):
        rows, pos = core_meta[c]
        if len(rows):
            out[rows] = res.results[c]["outd"][pos].astype(np.float32) + lin_b
    return out


last_results = None
last_inmaps = None
last_meta = None


# revision 22
# speedup vs baseline: 1.0786x; 1.0786x over previous
"""GCMCGraphConv Trainium2 kernel (8 NeuronCores, SPMD).

Design notes (v3):

Sharding: destination-partitioned edge parallelism. Edges are sorted by
edge_dst on the host; consecutive nonzero-degree dst rows are greedily
packed into groups of <=CAP edges and <=128 rows, and groups are dealt
round-robin to the 8 cores. Every group has a fixed capacity of CAP
slots (3 tiles of 128), so the SPMD program is identical across cores;
padding slots carry zero weights.

The per-edge table rows are pre-gathered ON THE HOST into dense per-slot
streams (the SWDGE indirect-DMA path costs ~1us of gpsimd time per 128
gathered rows on this target, which would dominate the kernel; dense
streams move the same bytes at full DMA bandwidth):
  - rth  [128, n_slots] bf16: review embedding rows, pre-TRANSPOSED so
    the MLP's first matmul can consume them directly (no on-chip
    transposes).
  - fth  [n_slots, 256] bf16: feat rows per slot, pre-scaled by
    wpa = sigmoid(rfeat@prob_w)*cj*ci on the host.
The other gating scalar wra = sigmoid(rfeat@score_w)*cj*ci ships as a
per-slot weight and scales the one-hot scatter matrix of the a2 path.

On-chip per tile (128 slots): MLP layer1 (bf16) + Gelu -> fp8e5, layer2
as ONE DoubleRow fp8 matmul (256-deep contraction in half the cycles) +
Gelu -> bf16. Layer3 (rw3) is NOT applied per edge: messages are
scatter-summed first and rw3 is folded into the final linear
(out_rf = (lin_w@rw3) @ G with G = sum_e wra_e * a2_e one-hot scattered,
which is exact because everything after the second Gelu is linear).

Scatter: per tile a plain one-hot S = onehot(dst) and its scaled copy
S_b = S*wra (one single-op DVE instruction each) feed four 128-col matmuls
accumulating hTa = sum wpa*feat (x) onehot and G = sum wra*a2 (x) onehot
in PSUM over the group's 3 tiles. Per group, four 256-col matmuls apply
lin_w / lin_w@rw3 to produce out[dst0:dst0+128, 256]; the bias and the
zero-degree rows are applied on the host during reassembly.
"""

import os

import numpy as np

P = 128
FEAT = 256
REV_DIM = 128
CAP = 384        # slots per group (3 tiles), <=128 dst rows per group
TPG = CAP // P   # tiles per group = 3
GB = 8           # groups per gather/stage batch
N_CORES = 8

_prog_cache = {}


def _build_program(ng):
    from concourse import tile, mybir, bacc

    n_slots = ng * CAP
    T = ng * TPG                 # tiles per core
    nb = ng // GB                # stage batches
    SLOTS_B = CAP * GB           # slots per batch
    TB = TPG * GB                # tiles per batch
    PAIRS_B = TB // 2
    f32 = mybir.dt.float32
    bf16 = mybir.dt.bfloat16
    f16 = mybir.dt.float16
    fp8 = mybir.dt.float8e5
    MM = mybir.MatmulPerfMode

    nc = bacc.Bacc(None, target_bir_lowering=False, debug=False)

    rth = nc.declare_dram_parameter("rth", [P, n_slots], bf16, isOutput=False)
    fth = nc.declare_dram_parameter("fth", [n_slots, FEAT], bf16, isOutput=False)
    wra = nc.declare_dram_parameter("wra", [P, T], f32, isOutput=False)
    dstr = nc.declare_dram_parameter("dstr", [P, T], f32, isOutput=False)
    rw1t = nc.declare_dram_parameter("rw1t", [REV_DIM, FEAT], bf16, isOutput=False)
    rw2t8 = nc.declare_dram_parameter("rw2t8", [P, 2, FEAT], fp8, isOutput=False)
    w3lt = nc.declare_dram_parameter("w3lt", [P, 2, FEAT], bf16, isOutput=False)
    lwt = nc.declare_dram_parameter("lwt", [P, 2, FEAT], bf16, isOutput=False)
    iota = nc.declare_dram_parameter("iota", [P, P], bf16, isOutput=False)
    outd = nc.declare_dram_parameter("outd", [ng * P, FEAT], bf16, isOutput=True)

    AF = mybir.ActivationFunctionType
    OP = mybir.AluOpType

    with tile.TileContext(nc) as tc:
        with tc.tile_pool(name="const", bufs=1) as cpool, \
             tc.tile_pool(name="stage", bufs=2) as stg, \
             tc.tile_pool(name="a1p", bufs=3) as a1pool, \
             tc.tile_pool(name="msg", bufs=8) as msg, \
             tc.tile_pool(name="drain", bufs=4) as drn, \
             tc.tile_pool(name="ot", bufs=2) as otp, \
             tc.tile_pool(name="psA", bufs=1, space="PSUM") as psA, \
             tc.tile_pool(name="psB", bufs=1, space="PSUM") as psB, \
             tc.tile_pool(name="psC", bufs=3, space="PSUM") as psC, \
             tc.tile_pool(name="psO", bufs=1, space="PSUM") as psO:

            c_rw1t = cpool.tile([REV_DIM, FEAT], bf16)
            nc.sync.dma_start(out=c_rw1t[:], in_=rw1t[:])
            c_rw2t8 = cpool.tile([P, 2, FEAT], fp8)
            nc.sync.dma_start(out=c_rw2t8[:], in_=rw2t8[:])
            c_w3lt = cpool.tile([P, 2, FEAT], bf16)
            nc.sync.dma_start(out=c_w3lt[:], in_=w3lt[:])
            c_lwt = cpool.tile([P, 2, FEAT], bf16)
            nc.sync.dma_start(out=c_lwt[:], in_=lwt[:])
            c_iota = cpool.tile([P, P], bf16)
            nc.sync.dma_start(out=c_iota[:], in_=iota[:])
            c_wra = cpool.tile([P, T], f32)
            nc.sync.dma_start(out=c_wra[:], in_=wra[:])
            c_dstr = cpool.tile([P, T], f32)
            nc.sync.dma_start(out=c_dstr[:], in_=dstr[:])

            for b in range(nb):
                s0 = b * SLOTS_B
                rts = stg.tile([P, SLOTS_B], bf16, tag="rts")
                nc.sync.dma_start(out=rts[:], in_=rth[:, s0:s0 + SLOTS_B])
                fts = stg.tile([P, TB, FEAT], bf16, tag="fts")
                nc.sync.dma_start(
                    out=fts[:],
                    in_=fth[s0:s0 + SLOTS_B, :].rearrange("(n p) d -> p n d", p=P))
                a2b = stg.tile([P, TB, FEAT], bf16, tag="a2b")

                def emit_quad(q):
                    # 4 tiles (512 slots) per emission: wide acts amortize
                    # the activation-engine init overhead; psA/psB are
                    # 2-bank quad tiles, single-buffered.
                    a1ps = psA.tile([P, 2, 2 * FEAT], f32, tag="a1ps")
                    for m in range(2):
                        nc.tensor.matmul(
                            out=a1ps[:, m, :],
                            lhsT=c_rw1t[:, m * P:(m + 1) * P],
                            rhs=rts[:, q * 512:(q + 1) * 512],
                            start=True, stop=True)
                    a1sb = a1pool.tile([P, 2, 2 * FEAT], fp8, tag="a1sb")
                    nc.scalar.activation(out=a1sb[:], in_=a1ps[:], func=AF.Gelu)
                    a2ps = psB.tile([P, 4, FEAT], f32, tag="a2ps")
                    for k in range(4):
                        nc.tensor.matmul(
                            out=a2ps[:, k, :],
                            lhsT=a1sb[:, :, k * P:(k + 1) * P],
                            rhs=c_rw2t8[:],
                            start=True, stop=True, perf_mode=MM.DoubleRow)
                    nc.scalar.activation(out=a2b[:, q * 4:(q + 1) * 4, :],
                                         in_=a2ps[:], func=AF.Gelu)

                po = None

                def emit_group(gi, po):
                    g = b * GB + gi
                    acc = psC.tile([P, 2, FEAT], f32, tag="acc")
                    # PSUM accumulation chains must be contiguous per bank
                    # region on HW: build all S tiles first, then run the
                    # four region chains back to back.
                    sas, scls = [], []
                    for gt in range(TPG):
                        tg = b * TB + gi * TPG + gt
                        ti = gi * TPG + gt
                        sa = msg.tile([P, P], bf16, tag="sa")
                        nc.vector.tensor_scalar(
                            out=sa[:], in0=c_iota[:],
                            scalar1=c_dstr[:, tg:tg + 1],
                            scalar2=None, op0=OP.is_equal)
                        sas.append(sa)
                        # scale a2 by wra once per tile (all-bf16, DVE 2x)
                        # instead of building a second scaled one-hot
                        scl = msg.tile([P, FEAT], bf16, tag="scl")
                        nc.vector.tensor_scalar(
                            out=scl[:], in0=a2b[:, ti, :],
                            scalar1=c_wra[:, tg:tg + 1],
                            scalar2=None, op0=OP.mult)
                        scls.append(scl)
                    for f in range(2):
                        for gt in range(TPG):
                            ti = gi * TPG + gt
                            nc.tensor.matmul(
                                out=acc[:, 0, f * P:(f + 1) * P],
                                lhsT=fts[:, ti, f * P:(f + 1) * P],
                                rhs=sas[gt][:],
                                start=(gt == 0), stop=(gt == TPG - 1))
                    for j in range(2):
                        for gt in range(TPG):
                            nc.tensor.matmul(
                                out=acc[:, 1, j * P:(j + 1) * P],
                                lhsT=scls[gt][:, j * P:(j + 1) * P],
                                rhs=sas[gt][:],
                                start=(gt == 0), stop=(gt == TPG - 1))
                    hg = drn.tile([P, 2, FEAT], bf16, tag="hg")
                    nc.vector.tensor_copy(out=hg[:], in_=acc[:])
                    hta = hg[:, 0, :]
                    gsb = hg[:, 1, :]
                    if gi % 2 == 0:
                        po = psO.tile([P, 2, FEAT], f32, tag="po")
                    for f in range(2):
                        nc.tensor.matmul(
                            out=po[:, gi % 2, :],
                            lhsT=hta[:, f * P:(f + 1) * P],
                            rhs=c_lwt[:, f, :],
                            start=(f == 0), stop=False)
                    for j in range(2):
                        nc.tensor.matmul(
                            out=po[:, gi % 2, :],
                            lhsT=gsb[:, j * P:(j + 1) * P],
                            rhs=c_w3lt[:, j, :],
                            start=False, stop=(j == 1))
                    if gi % 2 == 1:
                        ot = otp.tile([P, 2, FEAT], bf16, tag="ot")
                        nc.vector.tensor_copy(out=ot[:], in_=po[:])
                        g0 = g - 1
                        nc.sync.dma_start(
                            out=outd[g0 * P:(g0 + 2) * P, :].rearrange(
                                "(n p) d -> p n d", p=P),
                            in_=ot[:])
                    return po

                # interleave MLP quads with scatter groups: emit each
                # group as soon as its 3 tiles are computed
                gnext = 0
                for q in range(TB // 4):
                    emit_quad(q)
                    while gnext < GB and (gnext * TPG + TPG - 1) < 4 * (q + 1):
                        po = emit_group(gnext, po)
                        gnext += 1
                while gnext < GB:
                    po = emit_group(gnext, po)
                    gnext += 1
    nc.compile()
    return nc


def kernel(**inputs):
    import ml_dtypes
    from concourse.bass_utils import run_bass_kernel_spmd

    feat = np.asarray(inputs["feat"], dtype=np.float32)
    cj = np.asarray(inputs["cj"], dtype=np.float32)
    ci = np.asarray(inputs["ci"], dtype=np.float32)
    edge_src = np.asarray(inputs["edge_src"]).astype(np.int64)
    edge_dst = np.asarray(inputs["edge_dst"]).astype(np.int64)
    review_id = np.asarray(inputs["review_id"]).astype(np.int64)
    rev_emb = np.asarray(inputs["review_emb"], dtype=np.float32)
    prob_w = np.asarray(inputs["prob_w"], dtype=np.float32)
    score_w = np.asarray(inputs["score_w"], dtype=np.float32)
    rw1 = np.asarray(inputs["rw1"], dtype=np.float32)
    rw2 = np.asarray(inputs["rw2"], dtype=np.float32)
    rw3 = np.asarray(inputs["rw3"], dtype=np.float32)
    lin_w = np.asarray(inputs["lin_w"], dtype=np.float32)
    lin_b = np.asarray(inputs["lin_b"], dtype=np.float32)

    n_dst = ci.shape[0]
    bf = ml_dtypes.bfloat16

    order = np.argsort(edge_dst, kind="stable")
    s_src = edge_src[order]
    s_dst = edge_dst[order]
    s_rev = review_id[order]
    s_w = (cj[s_src, 0] * ci[s_dst, 0]).astype(np.float32)
    rfeat = rev_emb[s_rev]
    pa = 1.0 / (1.0 + np.exp(-(rfeat @ prob_w[0])))
    ra = 1.0 / (1.0 + np.exp(-(rfeat @ score_w[0])))
    s_wpa = (pa * s_w).astype(np.float32)
    s_wra = (ra * s_w).astype(np.float32)

    # nonzero dst rows in sorted order, with degree and edge offsets
    uniq_dst, deg = np.unique(s_dst, return_counts=True)
    row_end = np.cumsum(deg)            # edge offset after each row
    row_start = row_end - deg
    nrows_tot = len(uniq_dst)

    # greedy pack consecutive rows into groups: <=CAP edges, <=128 rows
    grp_row0 = []      # first row index (into uniq_dst) of each group
    grp_nrows = []
    i = 0
    while i < nrows_tot:
        base = row_start[i]
        # farthest j with row_end[j] - base <= CAP
        j = np.searchsorted(row_end, base + CAP, side="right") - 1
        j = min(max(j, i), i + P - 1)
        grp_row0.append(i)
        grp_nrows.append(j - i + 1)
        i = j + 1
    ngroups = len(grp_row0)
    ng = -(-ngroups // N_CORES)
    ng = -(-ng // GB) * GB             # groups per core, multiple of GB
    n_slots = ng * CAP
    T = ng * TPG

    # deal groups round-robin to cores: group g -> core g % 8, index g // 8
    rev16 = rev_emb.astype(bf)
    feat16 = feat.astype(bf)

    consts = dict(
        rw1t=np.ascontiguousarray(rw1.T).astype(bf),
        rw2t8=np.ascontiguousarray(
            rw2.T.reshape(2, P, FEAT).transpose(1, 0, 2)).astype(ml_dtypes.float8_e5m2),
        w3lt=np.ascontiguousarray(
            (lin_w @ rw3).T.reshape(2, P, FEAT).transpose(1, 0, 2)).astype(bf),
        lwt=np.ascontiguousarray(
            lin_w.T.reshape(2, P, FEAT).transpose(1, 0, 2)).astype(bf),
        iota=np.broadcast_to(np.arange(P), (P, P)).astype(bf).copy(),
    )

    in_maps = []
    core_meta = []
    for c in range(N_CORES):
        gl = list(range(c, ngroups, N_CORES))[:ng]
        slot_rev = np.zeros(n_slots, dtype=np.int64)
        slot_src = np.zeros(n_slots, dtype=np.int64)
        wpa_s = np.zeros(n_slots, dtype=np.float32)
        wra_s = np.zeros(n_slots, dtype=np.float32)
        dst_s = np.full(n_slots, -1.0, dtype=np.float32)
        rows_all = []
        pos_all = []
        for k, g in enumerate(gl):
            r0, nr = grp_row0[g], grp_nrows[g]
            e0, e1 = row_start[r0], row_end[r0 + nr - 1]
            n = e1 - e0
            s0 = k * CAP
            slot_rev[s0:s0 + n] = s_rev[e0:e1]
            slot_src[s0:s0 + n] = s_src[e0:e1]
            wpa_s[s0:s0 + n] = s_wpa[e0:e1]
            wra_s[s0:s0 + n] = s_wra[e0:e1]
            dst_s[s0:s0 + n] = (
                np.searchsorted(uniq_dst[r0:r0 + nr], s_dst[e0:e1])
            ).astype(np.float32)
            rows_all.append(uniq_dst[r0:r0 + nr])
            pos_all.append(k * P + np.arange(nr))
        rth = np.ascontiguousarray(rev16[slot_rev].T)           # [128, n_slots]
        fth = np.ascontiguousarray(
            (feat[slot_src] * wpa_s[:, None]).astype(bf))       # [n_slots, 256]
        im = dict(
            rth=rth, fth=fth,
            wra=np.ascontiguousarray(wra_s.reshape(T, P).T),
            dstr=np.ascontiguousarray(dst_s.reshape(T, P).T),
            **consts)
        in_maps.append(im)
        core_meta.append((np.concatenate(rows_all) if rows_all else np.zeros(0, dtype=np.int64),
                          np.concatenate(pos_all) if pos_all else np.zeros(0, dtype=np.int64)))

    global last_inmaps, last_meta
    last_inmaps = in_maps
    last_meta = dict(ng=ng, T=T, n_slots=n_slots, core_meta=core_meta,
                     uniq_dst=uniq_dst, grp_row0=grp_row0, grp_nrows=grp_nrows)

    if ng not in _prog_cache:
        _prog_cache[ng] = _build_program(ng)
    nc = _prog_cache[ng]

    trace = bool(os.environ.get("BASS_KERNEL_TRACE"))
    res = run_bass_kernel_spmd(nc, in_maps, core_ids=list(range(N_CORES)),
                               trace=trace)
    global last_results
    last_results = res

    out = np.broadcast_to(lin_b, (n_dst, FEAT)).astype(np.float32).copy()
    for c in range(N_CORES):
        rows, pos = core_meta[c]
        if len(rows):
            out[rows] = res.results[c]["outd"][pos].astype(np.float32) + lin_b
    return out


last_results = None
last_inmaps = None
last_meta = None


# revision 24
# speedup vs baseline: 1.1153x; 1.0340x over previous
"""GCMCGraphConv Trainium2 kernel (8 NeuronCores, SPMD).

Design notes (v3):

Sharding: destination-partitioned edge parallelism. Edges are sorted by
edge_dst on the host; consecutive nonzero-degree dst rows are greedily
packed into groups of <=CAP edges and <=128 rows, and groups are dealt
round-robin to the 8 cores. Every group has a fixed capacity of CAP
slots (3 tiles of 128), so the SPMD program is identical across cores;
padding slots carry zero weights.

The per-edge table rows are pre-gathered ON THE HOST into dense per-slot
streams (the SWDGE indirect-DMA path costs ~1us of gpsimd time per 128
gathered rows on this target, which would dominate the kernel; dense
streams move the same bytes at full DMA bandwidth):
  - rth  [128, n_slots] bf16: review embedding rows, pre-TRANSPOSED so
    the MLP's first matmul can consume them directly (no on-chip
    transposes).
  - fth  [n_slots, 256] bf16: feat rows per slot, pre-scaled by
    wpa = sigmoid(rfeat@prob_w)*cj*ci on the host.
The other gating scalar wra = sigmoid(rfeat@score_w)*cj*ci ships as a
per-slot weight and scales the one-hot scatter matrix of the a2 path.

On-chip per tile (128 slots): MLP layer1 (bf16) + Gelu -> fp8e5, layer2
as ONE DoubleRow fp8 matmul (256-deep contraction in half the cycles) +
Gelu -> bf16. Layer3 (rw3) is NOT applied per edge: messages are
scatter-summed first and rw3 is folded into the final linear
(out_rf = (lin_w@rw3) @ G with G = sum_e wra_e * a2_e one-hot scattered,
which is exact because everything after the second Gelu is linear).

Scatter: per tile a plain one-hot S = onehot(dst) and its scaled copy
S_b = S*wra (one single-op DVE instruction each) feed four 128-col matmuls
accumulating hTa = sum wpa*feat (x) onehot and G = sum wra*a2 (x) onehot
in PSUM over the group's 3 tiles. Per group, four 256-col matmuls apply
lin_w / lin_w@rw3 to produce out[dst0:dst0+128, 256]; the bias and the
zero-degree rows are applied on the host during reassembly.
"""

import os

import numpy as np

P = 128
FEAT = 256
REV_DIM = 128
CAP = 384        # slots per group (3 tiles), <=128 dst rows per group
TPG = CAP // P   # tiles per group = 3
GB = 8           # groups per gather/stage batch
N_CORES = 8

_prog_cache = {}


def _build_program(ng):
    from concourse import tile, mybir, bacc

    n_slots = ng * CAP
    T = ng * TPG                 # tiles per core
    nb = ng // GB                # stage batches
    SLOTS_B = CAP * GB           # slots per batch
    TB = TPG * GB                # tiles per batch
    PAIRS_B = TB // 2
    f32 = mybir.dt.float32
    bf16 = mybir.dt.bfloat16
    f16 = mybir.dt.float16
    fp8 = mybir.dt.float8e5
    MM = mybir.MatmulPerfMode

    nc = bacc.Bacc(None, target_bir_lowering=False, debug=False)

    rth = nc.declare_dram_parameter("rth", [P, n_slots], bf16, isOutput=False)
    fth = nc.declare_dram_parameter("fth", [n_slots, FEAT], bf16, isOutput=False)
    wra = nc.declare_dram_parameter("wra", [P, T], f32, isOutput=False)
    dstr = nc.declare_dram_parameter("dstr", [P, T], f32, isOutput=False)
    rw1t = nc.declare_dram_parameter("rw1t", [REV_DIM, FEAT], bf16, isOutput=False)
    rw2t8 = nc.declare_dram_parameter("rw2t8", [P, 2, FEAT], fp8, isOutput=False)
    w3lt = nc.declare_dram_parameter("w3lt", [P, 2, FEAT], bf16, isOutput=False)
    lwt = nc.declare_dram_parameter("lwt", [P, 2, FEAT], bf16, isOutput=False)
    iota = nc.declare_dram_parameter("iota", [P, P], bf16, isOutput=False)
    outd = nc.declare_dram_parameter("outd", [ng * P, FEAT], bf16, isOutput=True)

    AF = mybir.ActivationFunctionType
    OP = mybir.AluOpType

    with tile.TileContext(nc) as tc:
        with tc.tile_pool(name="const", bufs=1) as cpool, \
             tc.tile_pool(name="stage", bufs=3) as stg, \
             tc.tile_pool(name="a1p", bufs=3) as a1pool, \
             tc.tile_pool(name="msg", bufs=8) as msg, \
             tc.tile_pool(name="drain", bufs=4) as drn, \
             tc.tile_pool(name="ot", bufs=2) as otp, \
             tc.tile_pool(name="psA", bufs=1, space="PSUM") as psA, \
             tc.tile_pool(name="psB", bufs=1, space="PSUM") as psB, \
             tc.tile_pool(name="psC", bufs=3, space="PSUM") as psC, \
             tc.tile_pool(name="psO", bufs=1, space="PSUM") as psO:

            c_rw1t = cpool.tile([REV_DIM, FEAT], bf16)
            nc.sync.dma_start(out=c_rw1t[:], in_=rw1t[:])
            c_rw2t8 = cpool.tile([P, 2, FEAT], fp8)
            nc.sync.dma_start(out=c_rw2t8[:], in_=rw2t8[:])
            c_w3lt = cpool.tile([P, 2, FEAT], bf16)
            nc.sync.dma_start(out=c_w3lt[:], in_=w3lt[:])
            c_lwt = cpool.tile([P, 2, FEAT], bf16)
            nc.sync.dma_start(out=c_lwt[:], in_=lwt[:])
            c_iota = cpool.tile([P, P], bf16)
            nc.sync.dma_start(out=c_iota[:], in_=iota[:])
            c_wra = cpool.tile([P, T], f32)
            nc.sync.dma_start(out=c_wra[:], in_=wra[:])
            c_dstr = cpool.tile([P, T], f32)
            nc.sync.dma_start(out=c_dstr[:], in_=dstr[:])

            for b in range(nb):
                s0 = b * SLOTS_B
                rts = stg.tile([P, SLOTS_B], bf16, tag="rts")
                nc.sync.dma_start(out=rts[:], in_=rth[:, s0:s0 + SLOTS_B])
                fts = stg.tile([P, TB, FEAT], bf16, tag="fts")
                nc.sync.dma_start(
                    out=fts[:],
                    in_=fth[s0:s0 + SLOTS_B, :].rearrange("(n p) d -> p n d", p=P))
                a2b = stg.tile([P, TB, FEAT], bf16, tag="a2b")

                def emit_quad(q):
                    # 4 tiles (512 slots) per emission: wide acts amortize
                    # the activation-engine init overhead; psA/psB are
                    # 2-bank quad tiles, single-buffered.
                    a1ps = psA.tile([P, 2, 2 * FEAT], f32, tag="a1ps")
                    for m in range(2):
                        nc.tensor.matmul(
                            out=a1ps[:, m, :],
                            lhsT=c_rw1t[:, m * P:(m + 1) * P],
                            rhs=rts[:, q * 512:(q + 1) * 512],
                            start=True, stop=True)
                    a1sb = a1pool.tile([P, 2, 2 * FEAT], fp8, tag="a1sb")
                    nc.scalar.activation(out=a1sb[:], in_=a1ps[:], func=AF.Gelu)
                    a2ps = psB.tile([P, 4, FEAT], f32, tag="a2ps")
                    for k in range(4):
                        nc.tensor.matmul(
                            out=a2ps[:, k, :],
                            lhsT=a1sb[:, :, k * P:(k + 1) * P],
                            rhs=c_rw2t8[:],
                            start=True, stop=True, perf_mode=MM.DoubleRow)
                    nc.scalar.activation(out=a2b[:, q * 4:(q + 1) * 4, :],
                                         in_=a2ps[:], func=AF.Gelu)

                po = None

                def emit_group(gi, po):
                    g = b * GB + gi
                    acc = psC.tile([P, 2, FEAT], f32, tag="acc")
                    # PSUM accumulation chains must be contiguous per bank
                    # region on HW: build all S tiles first, then run the
                    # four region chains back to back.
                    sas, sbs = [], []
                    for gt in range(TPG):
                        tg = b * TB + gi * TPG + gt
                        sa = msg.tile([P, P], bf16, tag="sa")
                        nc.vector.tensor_scalar(
                            out=sa[:], in0=c_iota[:],
                            scalar1=c_dstr[:, tg:tg + 1],
                            scalar2=None, op0=OP.is_equal)
                        sas.append(sa)
                        sb_ = msg.tile([P, P], bf16, tag="sb")
                        nc.vector.tensor_scalar(
                            out=sb_[:], in0=sa[:],
                            scalar1=c_wra[:, tg:tg + 1],
                            scalar2=None, op0=OP.mult)
                        sbs.append(sb_)
                    for f in range(2):
                        for gt in range(TPG):
                            ti = gi * TPG + gt
                            nc.tensor.matmul(
                                out=acc[:, 0, f * P:(f + 1) * P],
                                lhsT=fts[:, ti, f * P:(f + 1) * P],
                                rhs=sas[gt][:],
                                start=(gt == 0), stop=(gt == TPG - 1))
                    for j in range(2):
                        for gt in range(TPG):
                            ti = gi * TPG + gt
                            nc.tensor.matmul(
                                out=acc[:, 1, j * P:(j + 1) * P],
                                lhsT=a2b[:, ti, j * P:(j + 1) * P],
                                rhs=sbs[gt][:],
                                start=(gt == 0), stop=(gt == TPG - 1))
                    hg = drn.tile([P, 2, FEAT], bf16, tag="hg")
                    nc.vector.tensor_copy(out=hg[:], in_=acc[:])
                    hta = hg[:, 0, :]
                    gsb = hg[:, 1, :]
                    if gi % 2 == 0:
                        po = psO.tile([P, 2, FEAT], f32, tag="po")
                    for f in range(2):
                        nc.tensor.matmul(
                            out=po[:, gi % 2, :],
                            lhsT=hta[:, f * P:(f + 1) * P],
                            rhs=c_lwt[:, f, :],
                            start=(f == 0), stop=False)
                    for j in range(2):
                        nc.tensor.matmul(
                            out=po[:, gi % 2, :],
                            lhsT=gsb[:, j * P:(j + 1) * P],
                            rhs=c_w3lt[:, j, :],
                            start=False, stop=(j == 1))
                    if gi % 2 == 1:
                        ot = otp.tile([P, 2, FEAT], bf16, tag="ot")
                        nc.vector.tensor_copy(out=ot[:], in_=po[:])
                        g0 = g - 1
                        nc.sync.dma_start(
                            out=outd[g0 * P:(g0 + 2) * P, :].rearrange(
                                "(n p) d -> p n d", p=P),
                            in_=ot[:])
                    return po

                # interleave MLP quads with scatter groups: emit each
                # group as soon as its 3 tiles are computed
                gnext = 0
                for q in range(TB // 4):
                    emit_quad(q)
                    while gnext < GB and (gnext * TPG + TPG - 1) < 4 * (q + 1):
                        po = emit_group(gnext, po)
                        gnext += 1
                while gnext < GB:
                    po = emit_group(gnext, po)
                    gnext += 1
    nc.compile()
    return nc


def kernel(**inputs):
    import ml_dtypes
    from concourse.bass_utils import run_bass_kernel_spmd

    feat = np.asarray(inputs["feat"], dtype=np.float32)
    cj = np.asarray(inputs["cj"], dtype=np.float32)
    ci = np.asarray(inputs["ci"], dtype=np.float32)
    edge_src = np.asarray(inputs["edge_src"]).astype(np.int64)
    edge_dst = np.asarray(inputs["edge_dst"]).astype(np.int64)
    review_id = np.asarray(inputs["review_id"]).astype(np.int64)
    rev_emb = np.asarray(inputs["review_emb"], dtype=np.float32)
    prob_w = np.asarray(inputs["prob_w"], dtype=np.float32)
    score_w = np.asarray(inputs["score_w"], dtype=np.float32)
    rw1 = np.asarray(inputs["rw1"], dtype=np.float32)
    rw2 = np.asarray(inputs["rw2"], dtype=np.float32)
    rw3 = np.asarray(inputs["rw3"], dtype=np.float32)
    lin_w = np.asarray(inputs["lin_w"], dtype=np.float32)
    lin_b = np.asarray(inputs["lin_b"], dtype=np.float32)

    n_dst = ci.shape[0]
    bf = ml_dtypes.bfloat16

    order = np.argsort(edge_dst, kind="stable")
    s_src = edge_src[order]
    s_dst = edge_dst[order]
    s_rev = review_id[order]
    s_w = (cj[s_src, 0] * ci[s_dst, 0]).astype(np.float32)
    rfeat = rev_emb[s_rev]
    pa = 1.0 / (1.0 + np.exp(-(rfeat @ prob_w[0])))
    ra = 1.0 / (1.0 + np.exp(-(rfeat @ score_w[0])))
    s_wpa = (pa * s_w).astype(np.float32)
    s_wra = (ra * s_w).astype(np.float32)

    # nonzero dst rows in sorted order, with degree and edge offsets
    uniq_dst, deg = np.unique(s_dst, return_counts=True)
    row_end = np.cumsum(deg)            # edge offset after each row
    row_start = row_end - deg
    nrows_tot = len(uniq_dst)

    # greedy pack consecutive rows into groups: <=CAP edges, <=128 rows
    grp_row0 = []      # first row index (into uniq_dst) of each group
    grp_nrows = []
    i = 0
    while i < nrows_tot:
        base = row_start[i]
        # farthest j with row_end[j] - base <= CAP
        j = np.searchsorted(row_end, base + CAP, side="right") - 1
        j = min(max(j, i), i + P - 1)
        grp_row0.append(i)
        grp_nrows.append(j - i + 1)
        i = j + 1
    ngroups = len(grp_row0)
    ng = -(-ngroups // N_CORES)
    ng = -(-ng // GB) * GB             # groups per core, multiple of GB
    n_slots = ng * CAP
    T = ng * TPG

    # deal groups round-robin to cores: group g -> core g % 8, index g // 8
    rev16 = rev_emb.astype(bf)
    feat16 = feat.astype(bf)

    consts = dict(
        rw1t=np.ascontiguousarray(rw1.T).astype(bf),
        rw2t8=np.ascontiguousarray(
            rw2.T.reshape(2, P, FEAT).transpose(1, 0, 2)).astype(ml_dtypes.float8_e5m2),
        w3lt=np.ascontiguousarray(
            (lin_w @ rw3).T.reshape(2, P, FEAT).transpose(1, 0, 2)).astype(bf),
        lwt=np.ascontiguousarray(
            lin_w.T.reshape(2, P, FEAT).transpose(1, 0, 2)).astype(bf),
        iota=np.broadcast_to(np.arange(P), (P, P)).astype(bf).copy(),
    )

    in_maps = []
    core_meta = []
    for c in range(N_CORES):
        gl = list(range(c, ngroups, N_CORES))[:ng]
        slot_rev = np.zeros(n_slots, dtype=np.int64)
        slot_src = np.zeros(n_slots, dtype=np.int64)
        wpa_s = np.zeros(n_slots, dtype=np.float32)
        wra_s = np.zeros(n_slots, dtype=np.float32)
        dst_s = np.full(n_slots, -1.0, dtype=np.float32)
        rows_all = []
        pos_all = []
        for k, g in enumerate(gl):
            r0, nr = grp_row0[g], grp_nrows[g]
            e0, e1 = row_start[r0], row_end[r0 + nr - 1]
            n = e1 - e0
            s0 = k * CAP
            slot_rev[s0:s0 + n] = s_rev[e0:e1]
            slot_src[s0:s0 + n] = s_src[e0:e1]
            wpa_s[s0:s0 + n] = s_wpa[e0:e1]
            wra_s[s0:s0 + n] = s_wra[e0:e1]
            dst_s[s0:s0 + n] = (
                np.searchsorted(uniq_dst[r0:r0 + nr], s_dst[e0:e1])
            ).astype(np.float32)
            rows_all.append(uniq_dst[r0:r0 + nr])
            pos_all.append(k * P + np.arange(nr))
        rth = np.ascontiguousarray(rev16[slot_rev].T)           # [128, n_slots]
        fth = np.ascontiguousarray(
            (feat[slot_src] * wpa_s[:, None]).astype(bf))       # [n_slots, 256]
        im = dict(
            rth=rth, fth=fth,
            wra=np.ascontiguousarray(wra_s.reshape(T, P).T),
            dstr=np.ascontiguousarray(dst_s.reshape(T, P).T),
            **consts)
        in_maps.append(im)
        core_meta.append((np.concatenate(rows_all) if rows_all else np.zeros(0, dtype=np.int64),
                          np.concatenate(pos_all) if pos_all else np.zeros(0, dtype=np.int64)))

    global last_inmaps, last_meta
    last_inmaps = in_maps
    last_meta = dict(ng=ng, T=T, n_slots=n_slots, core_meta=core_meta,
                     uniq_dst=uniq_dst, grp_row0=grp_row0, grp_nrows=grp_nrows)

    if ng not in _prog_cache:
        _prog_cache[ng] = _build_program(ng)
    nc = _prog_cache[ng]

    trace = bool(os.environ.get("BASS_KERNEL_TRACE"))
    res = run_bass_kernel_spmd(nc, in_maps, core_ids=list(range(N_CORES)),
                               trace=trace)
    global last_results
    last_results = res

    out = np.broadcast_to(lin_b, (n_dst, FEAT)).astype(np.float32).copy()
    for c in range(N_CORES):
        rows, pos = core_meta[c]
        if len(rows):
            out[rows] = res.results[c]["outd"][pos].astype(np.float32) + lin_b
    return out


last_results = None
last_inmaps = None
last_meta = None
